# revision 12
# baseline (speedup 1.0000x reference)
"""Trainium2 Bass kernel for nn_KGVAE (2-layer BDD RelGraphConv + gaussian sample).

Strategy (8 NeuronCores, SPMD):
  - Nodes are range-sharded by id across the 8 cores (12500 each, padded to
    12544 = 98 blocks of 128).  Each core owns all edges whose dst lands in
    its node range, so per-layer aggregates need no cross-core reduction.
  - Within a core, nodes are re-bucketed into 128-node blocks by LPT on
    in-degree so every block sees a near-equal number of incoming edges.
  - Layer compute is two phases over a DRAM message buffer (bf16):
      Phase A (edges ordered src-chunk-major, relation-minor): batched
        dma_gather(transpose=True) pulls h[src]^T tiles straight into SBUF
        (int16 indices, chunk-relative), one matmul per 128-edge tile
        against the relation's expanded block-diagonal weight, per-edge
        norm scaling fused into the PSUM->SBUF copy, then plain sequential
        DMA of the message tiles to the buffer (no scatter).
      Phase B (per 128-node dst block): batched dma_gather pulls the
        block's messages (msg-chunk-relative indices) as [128, t, F]
        tiles, a one-hot slot matrix built on-device (iota + is_equal)
        aggregates them into PSUM along with the self-loop matmul;
        epilogue adds bias and applies the activation / gaussian head.
  - All bulk tensors are bf16; matmuls bf16 with fp32 PSUM; epilogues fp32.
  - Two SPMD dispatches; the host concatenates per-core layer-1 outputs
    into the permuted full feature matrix layer 2 gathers from.

Self-contained: all shapes derive from the passed-in arrays.
"""

import heapq
import sys
import time
from contextlib import ExitStack

import ml_dtypes
import numpy as np

import concourse.mybir as mybir
import concourse.tile as tile
from concourse import bacc
from concourse import bass2jax as _b2j
from concourse import library_config

P = 128
NCORES = 8
SC = 25088      # src-chunk rows (int16-addressable, multiple of 128)
NSC = 4         # src chunks; NSC*SC = 100352 = padded hsrc rows
MC = 32640      # msgbuf chunk rows (multiple of 128, < 32768)
GA = 4          # phase-A tiles per transpose-gather op (512 idxs; 2 descs/idx)
SCRATCH = 32768  # SWDGE descriptor carveout bytes/partition
NQ = 4          # SWDGE queues
TRACE = False   # set True by a harness to collect exec times
TIME_ITERS = 9
LAST_EXEC_NS = []

F32 = mybir.dt.float32
BF16 = mybir.dt.bfloat16
I16 = mybir.dt.int16
NPBF16 = ml_dtypes.bfloat16
AF = mybir.ActivationFunctionType
ALU = mybir.AluOpType


def _cdiv(a, b):
    return -(-a // b)


def _expand_bd(W):
    """(R, NB, si, so) block weights -> (R, NB*si, NB*so) dense block-diagonal."""
    R, NB, si, so = W.shape
    out = np.zeros((R, NB * si, NB * so), dtype=np.float32)
    for b in range(NB):
        out[:, b * si:(b + 1) * si, b * so:(b + 1) * so] = W[:, b]
    return out


def _rank_within_group(sorted_keys):
    n = len(sorted_keys)
    if n == 0:
        return np.zeros(0, dtype=np.int64)
    starts = np.searchsorted(sorted_keys, sorted_keys, side="left")
    return np.arange(n, dtype=np.int64) - starts


def _wrap_idx(stream):
    """int idx stream (len % 128 == 0) -> [128, n//16] int16 wrapped layout."""
    n = len(stream)
    w = stream.astype(np.int16).reshape(n // 16, 16).T
    return np.ascontiguousarray(np.tile(w, (8, 1)))


def _plan(src, dst, etype, norm, N, R):
    E = len(src)
    NPC = N // NCORES
    assert NPC * NCORES == N
    NPAD = _cdiv(NPC, P) * P
    NBLK = NPAD // P
    assert NCORES * NPAD == NSC * SC

    deg = np.bincount(dst, minlength=N)

    # --- LPT assignment of each core's nodes into 128-node blocks ---
    block_of = np.empty(N, np.int32)
    slot_of = np.empty(N, np.int32)
    orig_of_slot = np.full((NCORES, NPAD), -1, np.int64)
    for c in range(NCORES):
        ids = np.arange(c * NPC, (c + 1) * NPC, dtype=np.int64)
        d = deg[ids]
        order = np.argsort(-d, kind="stable")
        heap = [(0, q) for q in range(NBLK)]
        heapq.heapify(heap)
        counts = np.zeros(NBLK, np.int32)
        for i in order:
            while True:
                load, q = heapq.heappop(heap)
                if counts[q] < P:
                    break
            node = ids[i]
            block_of[node] = q
            slot_of[node] = counts[q]
            orig_of_slot[c, q * P + counts[q]] = node
            counts[q] += 1
            heapq.heappush(heap, (load + int(d[i]), q))

    pg = np.empty(N, np.int64)
    for c in range(NCORES):
        ids = np.arange(c * NPC, (c + 1) * NPC, dtype=np.int64)
        pg[ids] = c * NPAD + block_of[ids].astype(np.int64) * P + slot_of[ids]

    core_of_edge = dst // NPC
    eidx_by_core = [np.nonzero(core_of_edge == c)[0] for c in range(NCORES)]

    def layer_tables(use_pg):
        # per-edge global gather row
        gsrc = [(pg[src[e]] if use_pg else src[e]) for e in eidx_by_core]
        rel = [etype[e] for e in eidx_by_core]
        nrm = [norm[e].reshape(-1).astype(np.float32) for e in eidx_by_core]
        qb = [block_of[dst[e]].astype(np.int64) for e in eidx_by_core]
        pb = [slot_of[dst[e]].astype(np.int64) for e in eidx_by_core]

        # ---- A side ----
        cnt = np.zeros((NCORES, NSC * R), np.int64)
        for c in range(NCORES):
            key = (gsrc[c] // SC) * R + rel[c]
            cnt[c] = np.bincount(key, minlength=NSC * R)
        T2 = _cdiv(cnt.max(axis=0), P)          # tiles per (ch, rel)
        A2_tiles = int(T2.sum())
        A2_slots = A2_tiles * P
        goff = np.concatenate([[0], np.cumsum(T2)]) * P  # slot offset per group

        tiles_rel = []                           # compile-time (ch, rel) per tile
        for g in range(NSC * R):
            tiles_rel += [(g // R, g % R)] * int(T2[g])

        per_core = []
        for c in range(NCORES):
            key = (gsrc[c] // SC) * R + rel[c]
            order = np.argsort(key, kind="stable")
            rank = _rank_within_group(key[order])
            apos = np.empty(len(key), np.int64)
            apos[order] = goff[key[order]] + rank

            idxA = np.zeros(A2_slots, np.int64)
            normA = np.zeros(A2_slots, np.float32)
            idxA[apos] = gsrc[c] - (gsrc[c] // SC) * SC
            normA[apos] = nrm[c]

            # ---- B side ----
            mch = apos // MC
            per_core.append(dict(apos=apos, mch=mch, idxA=idxA, normA=normA))

        NMC = _cdiv(A2_slots, MC)
        cntb = np.zeros((NCORES, NBLK * NMC), np.int64)
        for c in range(NCORES):
            pc = per_core[c]
            keyb = qb[c] * NMC + pc["mch"]
            cntb[c] = np.bincount(keyb, minlength=NBLK * NMC)
        TB2 = _cdiv(cntb.max(axis=0), P)        # tiles per (q, mch)
        B2_tiles = int(TB2.sum())
        B2_slots = B2_tiles * P
        boff = np.concatenate([[0], np.cumsum(TB2)]) * P

        for c in range(NCORES):
            pc = per_core[c]
            keyb = qb[c] * NMC + pc["mch"]
            order = np.argsort(keyb, kind="stable")
            rank = _rank_within_group(keyb[order])
            bpos = np.empty(len(keyb), np.int64)
            bpos[order] = boff[keyb[order]] + rank

            idxB = np.zeros(B2_slots, np.int64)
            dstp = np.full(B2_slots, 200.0, np.float32)
            idxB[bpos] = pc["apos"] - pc["mch"] * MC
            dstp[bpos] = pb[c].astype(np.float32)

            per_core[c] = dict(
                idxA_w=_wrap_idx(pc["idxA"]),
                normA_t=np.ascontiguousarray(
                    pc["normA"].reshape(A2_tiles, P).T),
                idxB_w=_wrap_idx(idxB),
                dstp_t=np.ascontiguousarray(dstp.reshape(B2_tiles, P).T),
            )

        return dict(
            T2=T2.reshape(NSC, R), TB2=TB2.reshape(NBLK, NMC),
            A2_tiles=A2_tiles, A2_slots=A2_slots,
            B2_tiles=B2_tiles, B2_slots=B2_slots, NMC=NMC,
            tiles_rel=tiles_rel, per_core=per_core,
        )

    return dict(
        NPC=NPC, NPAD=NPAD, NBLK=NBLK, R=R,
        orig_of_slot=orig_of_slot, pg=pg,
        L1=layer_tables(False), L2=layer_tables(True),
    )


def _build_layer(plan, lt, F, gauss):
    """Build one SPMD dispatch. lt = layer tables (plan['L1'|'L2'])."""
    NPAD, NBLK, R = plan["NPAD"], plan["NBLK"], plan["R"]
    T2, TB2, NMC = lt["T2"], lt["TB2"], lt["NMC"]
    A2_tiles, A2_slots = lt["A2_tiles"], lt["A2_slots"]
    B2_tiles, B2_slots = lt["B2_tiles"], lt["B2_slots"]
    tiles_rel = lt["tiles_rel"]
    H = 128
    GB = 8 if F == H else 4  # B-gather op cap (ring capacity, 512B rows)

    nc = bacc.Bacc("TRN2", target_bir_lowering=False, debug=False,
                   num_swdge_queues=NQ, dynamic_dma_scratch_size=SCRATCH)

    hsrc = nc.dram_tensor("hsrc", [NSC * SC, H], BF16, kind="ExternalInput")
    Wd = nc.dram_tensor("W", [R, H, F], BF16, kind="ExternalInput")
    loopw = nc.dram_tensor("loopw", [H, F], BF16, kind="ExternalInput")
    biasb = nc.dram_tensor("biasb", [P, F], F32, kind="ExternalInput")
    idxA_d = nc.dram_tensor("idxA", [P, A2_slots // 16], I16, kind="ExternalInput")
    normA_d = nc.dram_tensor("normA", [P, A2_tiles], F32, kind="ExternalInput")
    idxB_d = nc.dram_tensor("idxB", [P, B2_slots // 16], I16, kind="ExternalInput")
    dstp_d = nc.dram_tensor("dstp", [P, B2_tiles], F32, kind="ExternalInput")
    hloc = nc.dram_tensor("hloc", [NPAD, H], BF16, kind="ExternalInput")
    iota_d = nc.dram_tensor("iota", [P, P], F32, kind="ExternalInput")
    if gauss:
        epsl = nc.dram_tensor("epsl", [NPAD, H], F32, kind="ExternalInput")
        out_d = nc.dram_tensor("out", [NPAD, H], F32, kind="ExternalOutput")
    else:
        out_d = nc.dram_tensor("out", [NPAD, F], BF16, kind="ExternalOutput")
    msgbuf = nc.dram_tensor("msgbuf", [A2_slots, F], BF16)

    with tile.TileContext(nc) as tc, ExitStack() as ctx:
        nc.gpsimd.load_library(library_config.mlp)
        const = ctx.enter_context(tc.tile_pool(name="const", bufs=1))
        gpool = ctx.enter_context(tc.tile_pool(name="gpool", bufs=3))
        spool = ctx.enter_context(tc.tile_pool(name="spool", bufs=3))
        wpool = ctx.enter_context(tc.tile_pool(name="wpool", bufs=3))
        bpool = ctx.enter_context(tc.tile_pool(name="bpool", bufs=4))
        papool = ctx.enter_context(tc.tile_pool(name="papool", bufs=3, space="PSUM"))
        ptpool = ctx.enter_context(tc.tile_pool(name="ptpool", bufs=2, space="PSUM"))
        pbpool = ctx.enter_context(tc.tile_pool(name="pbpool", bufs=2, space="PSUM"))

        ident = const.tile([P, P], BF16)
        iota_sb = const.tile([P, P], F32)
        loopw_sb = const.tile([H, F], BF16)
        bias_sb = const.tile([P, F], F32)
        idxA_sb = const.tile([P, A2_slots // 16], I16)
        normA_sb = const.tile([P, A2_tiles], F32)
        idxB_sb = const.tile([P, B2_slots // 16], I16)
        dstp_sb = const.tile([P, B2_tiles], F32)
        nc.sync.dma_start(out=iota_sb[:], in_=iota_d[:])
        nc.sync.dma_start(out=loopw_sb[:], in_=loopw[:])
        nc.sync.dma_start(out=bias_sb[:], in_=biasb[:])
        nc.sync.dma_start(out=idxA_sb[:], in_=idxA_d[:])
        nc.sync.dma_start(out=normA_sb[:], in_=normA_d[:])
        nc.sync.dma_start(out=idxB_sb[:], in_=idxB_d[:])
        nc.sync.dma_start(out=dstp_sb[:], in_=dstp_d[:])
        from concourse.masks import make_identity
        make_identity(nc, ident[:])
        if gauss:
            eps_bias = const.tile([P, 1], F32)
            nc.vector.memset(eps_bias[:], 1e-8)
            one_bias = const.tile([P, 1], F32)
            nc.vector.memset(one_bias[:], 1.0)

        # ---------------- phase A ----------------
        # gather ops: runs of <= GA tiles within one src chunk
        ops = []
        k = 0
        for ch in range(NSC):
            nt_ch = int(T2[ch].sum())
            t = 0
            while t < nt_ch:
                nt = min(GA, nt_ch - t)
                ops.append((ch, k, nt))
                t += nt
                k += nt
        assert k == A2_tiles

        qrot = 0
        cur_rel = None
        w_sb = None
        for ch, k0, nt in ops:
            ht = gpool.tile([P, nt * P], BF16, tag="ht")
            nc.gpsimd.dma_gather(
                ht[:].rearrange("p (o n) -> p o n", o=1),
                hsrc[ch * SC:(ch + 1) * SC, :],
                idxA_sb[:, k0 * 8:(k0 + nt) * 8],
                nt * P, nt * P, H, transpose=True, queue_num=qrot,
            )
            qrot = (qrot + 1) % NQ
            stage = spool.tile([P, nt * F], BF16, tag="stage")
            for t in range(nt):
                kt = k0 + t
                r = tiles_rel[kt][1]
                if cur_rel != (ch, r):
                    w_sb = wpool.tile([H, F], BF16, tag="w")
                    nc.scalar.dma_start(out=w_sb[:], in_=Wd[r])
                    cur_rel = (ch, r)
                msg_ps = papool.tile([P, F], F32, tag="msg_ps")
                nc.tensor.matmul(out=msg_ps[:], lhsT=ht[:, t * P:(t + 1) * P],
                                 rhs=w_sb[:], start=True, stop=True)
                nc.vector.tensor_scalar(
                    out=stage[:, t * F:(t + 1) * F], in0=msg_ps[:],
                    scalar1=normA_sb[:, kt:kt + 1], scalar2=None, op0=ALU.mult,
                )
            nc.sync.dma_start(
                out=msgbuf[k0 * P:(k0 + nt) * P, :].rearrange(
                    "(t p) f -> p t f", p=P),
                in_=stage[:].rearrange("p (t f) -> p t f", f=F),
            )

        tc.strict_bb_all_engine_barrier()

        # ---------------- phase B ----------------
        kb = 0
        for q in range(NBLK):
            TBq = int(TB2[q].sum())
            out_ps = pbpool.tile([P, F], F32, tag="out_ps")
            hl_t = bpool.tile([P, H], BF16, tag="hl")
            nc.sync.dma_start(out=hl_t[:], in_=hloc[q * P:(q + 1) * P, :])
            hlT_ps = ptpool.tile([P, P], BF16, tag="hT_ps")
            nc.tensor.transpose(out=hlT_ps[:], in_=hl_t[:], identity=ident[:])
            hlT_sb = bpool.tile([P, P], BF16, tag="hT")
            nc.scalar.activation(out=hlT_sb[:], in_=hlT_ps[:], func=AF.Copy)
            nc.tensor.matmul(out=out_ps[:], lhsT=hlT_sb[:], rhs=loopw_sb[:],
                             start=True, stop=(TBq == 0))

            done = 0
            for mch in range(NMC):
                ntq = int(TB2[q][mch])
                t0 = 0
                while t0 < ntq:
                    nt = min(GB, ntq - t0)
                    mblk = bpool.tile([P, nt * F], BF16, tag="mblk")
                    nrows = min(MC, A2_slots - mch * MC)
                    nc.gpsimd.dma_gather(
                        mblk[:].rearrange("p (t f) -> p t f", f=F),
                        msgbuf[mch * MC:mch * MC + nrows, :],
                        idxB_sb[:, (kb + done) * 8:(kb + done + nt) * 8],
                        nt * P, nt * P, F, queue_num=qrot,
                    )
                    qrot = (qrot + 1) % NQ
                    for t in range(nt):
                        ktb = kb + done + t
                        P_t = bpool.tile([P, P], BF16, tag="Pt")
                        nc.vector.tensor_scalar(
                            out=P_t[:], in0=iota_sb[:],
                            scalar1=dstp_sb[:, ktb:ktb + 1], scalar2=None,
                            op0=ALU.is_equal,
                        )
                        nc.tensor.matmul(
                            out=out_ps[:], lhsT=P_t[:],
                            rhs=mblk[:, t * F:(t + 1) * F],
                            start=False, stop=(done + t == TBq - 1),
                        )
                    done += nt
                    t0 += nt
            kb += TBq

            hb = bpool.tile([P, F], F32, tag="hb")
            nc.vector.tensor_tensor(out=hb[:], in0=out_ps[:], in1=bias_sb[:],
                                    op=ALU.add)
            if not gauss:
                hbb = bpool.tile([P, F], BF16, tag="hbb")
                nc.scalar.activation(out=hbb[:], in_=hb[:], func=AF.Relu)
                nc.sync.dma_start(out=out_d[q * P:(q + 1) * P, :], in_=hbb[:])
            else:
                # softplus(x) = relu(x) + ln(1 + exp(-|x|)); sqrt(v) = exp(ln(v)/2)
                sq = bpool.tile([P, H], F32, tag="sq")
                ax = bpool.tile([P, H], F32, tag="ax")
                nc.scalar.activation(out=ax[:], in_=hb[:, H:2 * H], func=AF.Abs)
                nc.scalar.activation(out=ax[:], in_=ax[:], func=AF.Exp, scale=-1.0)
                nc.scalar.activation(out=ax[:], in_=ax[:], func=AF.Ln,
                                     bias=one_bias[:])
                nc.scalar.activation(out=sq[:], in_=hb[:, H:2 * H], func=AF.Relu)
                nc.vector.tensor_tensor(out=sq[:], in0=sq[:], in1=ax[:], op=ALU.add)
                nc.scalar.activation(out=sq[:], in_=sq[:], func=AF.Ln,
                                     bias=eps_bias[:])
                nc.scalar.activation(out=sq[:], in_=sq[:], func=AF.Exp, scale=0.5)
                ep = bpool.tile([P, H], F32, tag="ep")
                nc.sync.dma_start(out=ep[:], in_=epsl[q * P:(q + 1) * P, :])
                z_t = bpool.tile([P, H], F32, tag="z")
                nc.vector.tensor_tensor(out=z_t[:], in0=sq[:], in1=ep[:], op=ALU.mult)
                nc.vector.tensor_tensor(out=z_t[:], in0=z_t[:], in1=hb[:, :H],
                                        op=ALU.add)
                nc.sync.dma_start(out=out_d[q * P:(q + 1) * P, :], in_=z_t[:])
        assert kb == B2_tiles

    nc.compile()
    return nc


def _log(msg):
    print(f"[kernel] {msg}", file=sys.stderr, flush=True)


class _SpmdExec:
    """Compile a bass module into one persistent sharded PJRT executable.

    Inputs are staged onto the 8 devices once (device_put, untimed); runs
    reuse the staged arrays."""

    def __init__(self, nc, n_cores):
        import jax

        _b2j.install_neuronx_cc_hook()
        self.nc = nc
        self.n_cores = n_cores
        partition_name = (
            nc.partition_id_tensor.name if nc.partition_id_tensor else None
        )
        in_names, out_names, out_avals, zero_info = [], [], [], []
        for alloc in nc.m.functions[0].allocations:
            if not isinstance(alloc, mybir.MemoryLocationSet):
                continue
            name = alloc.memorylocations[0].name
            if alloc.kind == "ExternalInput":
                if name != partition_name:
                    in_names.append(name)
            elif alloc.kind == "ExternalOutput":
                out_names.append(name)
                shape = tuple(alloc.tensor_shape)
                dtype = mybir.dt.np(alloc.dtype)
                out_avals.append(jax.core.ShapedArray(shape, dtype))
                zero_info.append((shape, dtype))
        assert nc.dbg_addr is None, "build with debug=False"
        self.in_names = list(in_names)
        self.out_names = out_names
        n_params = len(in_names)
        n_outs = len(out_names)
        in_names = in_names + out_names
        if partition_name is not None:
            in_names.append(partition_name)

        def _body(*args):
            operands = list(args)
            if partition_name is not None:
                operands.append(_b2j.partition_id_tensor())
            outs = _b2j._bass_exec_p.bind(
                *operands,
                out_avals=tuple(out_avals),
                in_names=tuple(in_names),
                out_names=tuple(out_names),
                lowering_input_output_aliases=(),
                sim_require_finite=True,
                sim_require_nnan=True,
                nc=nc,
            )
            return tuple(outs)

        from jax.experimental.shard_map import shard_map
        from jax.sharding import Mesh, NamedSharding, PartitionSpec

        devices = jax.devices()[:n_cores]
        mesh = Mesh(np.asarray(devices), ("core",))
        self.mesh = mesh
        self.sharding = NamedSharding(mesh, PartitionSpec("core"))
        in_specs = (PartitionSpec("core"),) * (n_params + n_outs)
        out_specs = (PartitionSpec("core"),) * n_outs
        donate = tuple(range(n_params, n_params + n_outs))
        self.sharded = jax.jit(
            shard_map(_body, mesh=mesh, in_specs=in_specs,
                      out_specs=out_specs, check_rep=False),
            donate_argnums=donate, keep_unused=True,
        )
        import jax.numpy as jnp

        def _mk():
            return tuple(
                jnp.zeros((n_cores * s[0], *s[1:]), d) for s, d in zero_info
            )

        self._mkzeros = jax.jit(
            _mk, out_shardings=(self.sharding,) * n_outs)
        self.out_avals = out_avals

    def stage(self, in_maps):
        import jax

        staged = []
        for name in self.in_names:
            arr = np.concatenate(
                [np.asarray(m[name]) for m in in_maps], axis=0)
            staged.append(jax.device_put(arr, self.sharding))
        jax.block_until_ready(staged)
        return staged

    def run(self, staged):
        import jax

        zs = self._mkzeros()
        outs = self.sharded(*staged, *zs)
        jax.block_until_ready(outs)
        res = []
        for c in range(self.n_cores):
            d = {}
            for i, name in enumerate(self.out_names):
                g = np.asarray(outs[i])
                d[name] = g.reshape(self.n_cores, *self.out_avals[i].shape)[c]
            res.append(d)
        for o in outs:
            o.delete()
        return res

    def time_exec(self, staged, iters):
        """Steady-state per-dispatch device time: one warm call (T1), then
        `iters` back-to-back pipelined calls; the marginal spacing
        (Tk - T1)/(k-1) bounds device execution while excluding the
        client<->terminal round-trip latency a single blocking call pays."""
        import jax

        zs_all = [self._mkzeros() for _ in range(iters + 1)]
        jax.block_until_ready(zs_all)
        t0 = time.perf_counter()
        out1 = self.sharded(*staged, *zs_all[0])
        jax.block_until_ready(out1)
        t1 = time.perf_counter()
        outs = [self.sharded(*staged, *zs_all[1 + i]) for i in range(iters)]
        jax.block_until_ready(outs)
        t2 = time.perf_counter()
        single = t1 - t0
        marginal = (t2 - t1 - (single if iters > 1 else 0)) / max(iters - 1, 1)
        _log(f"  single-shot {single * 1e3:.2f} ms, pipelined x{iters} "
             f"{(t2 - t1) * 1e3:.2f} ms -> marginal {marginal * 1e3:.2f} ms")
        for o in outs:
            for x in o:
                x.delete()
        for x in out1:
            x.delete()
        return int(marginal * 1e9)


def kernel(node_ids, src, dst, etype, norm, emb, W1, loop1, b1, W2, loop2, b2, eps):
    node_ids = np.asarray(node_ids).astype(np.int64)
    src = np.asarray(src).astype(np.int64)
    dst = np.asarray(dst).astype(np.int64)
    etype = np.asarray(etype).astype(np.int64)
    norm = np.asarray(norm, np.float32)
    emb = np.asarray(emb, np.float32)
    W1 = np.asarray(W1, np.float32)
    loop1 = np.asarray(loop1, np.float32)
    b1 = np.asarray(b1, np.float32)
    W2 = np.asarray(W2, np.float32)
    loop2 = np.asarray(loop2, np.float32)
    b2 = np.asarray(b2, np.float32)
    eps = np.asarray(eps, np.float32)

    N, H = emb.shape
    R = W1.shape[0]
    F2 = W2.shape[1] * W2.shape[3]
    assert H == 128

    h0 = emb[node_ids]
    t0 = time.time()
    plan = _plan(src, dst, etype, norm, N, R)
    _log(f"plan {time.time() - t0:.1f}s "
         f"L1 A2={plan['L1']['A2_tiles']} B2={plan['L1']['B2_tiles']} "
         f"L2 A2={plan['L2']['A2_tiles']} B2={plan['L2']['B2_tiles']}")
    NPAD = plan["NPAD"]
    orig_of_slot = plan["orig_of_slot"]

    h0b = np.zeros((NSC * SC, H), NPBF16)
    h0b[:N] = h0.astype(NPBF16)
    W1bd = _expand_bd(W1).astype(NPBF16)
    W2bd = _expand_bd(W2).astype(NPBF16)
    iota = np.tile(np.arange(P, dtype=np.float32), (P, 1))
    bias1b = np.tile(b1.astype(np.float32), (P, 1))
    bias2b = np.tile(b2.astype(np.float32), (P, 1))

    hloc_c, eps_c = [], []
    for c in range(NCORES):
        sl = orig_of_slot[c]
        valid = sl >= 0
        hl = np.zeros((NPAD, H), NPBF16)
        hl[valid] = h0b[sl[valid]]
        ev = np.zeros((NPAD, H), np.float32)
        ev[valid] = eps[sl[valid]]
        hloc_c.append(hl)
        eps_c.append(ev)

    # ---- dispatch 1 ----
    lt1 = plan["L1"]
    t0 = time.time()
    nc1 = _build_layer(plan, lt1, F=H, gauss=False)
    _log(f"build1 {time.time() - t0:.1f}s")
    in_maps1 = []
    for c in range(NCORES):
        pc = lt1["per_core"][c]
        in_maps1.append(dict(
            hsrc=h0b, W=W1bd, loopw=loop1.astype(NPBF16),
            biasb=bias1b, idxA=pc["idxA_w"], normA=pc["normA_t"],
            idxB=pc["idxB_w"], dstp=pc["dstp_t"], hloc=hloc_c[c], iota=iota,
        ))
    t0 = time.time()
    ex1 = _SpmdExec(nc1, NCORES)
    staged1 = ex1.stage(in_maps1)
    _log(f"stage1 {time.time() - t0:.1f}s")
    t0 = time.time()
    res1 = ex1.run(staged1)
    _log(f"layer1 run {time.time() - t0:.1f}s")
    h1full = np.concatenate([res1[c]["out"] for c in range(NCORES)], axis=0)

    # ---- dispatch 2 ----
    lt2 = plan["L2"]
    t0 = time.time()
    nc2 = _build_layer(plan, lt2, F=F2, gauss=True)
    _log(f"build2 {time.time() - t0:.1f}s")
    in_maps2 = []
    for c in range(NCORES):
        pc = lt2["per_core"][c]
        in_maps2.append(dict(
            hsrc=h1full, W=W2bd, loopw=loop2.astype(NPBF16),
            biasb=bias2b, idxA=pc["idxA_w"], normA=pc["normA_t"],
            idxB=pc["idxB_w"], dstp=pc["dstp_t"],
            hloc=h1full[c * NPAD:(c + 1) * NPAD], iota=iota, epsl=eps_c[c],
        ))
    t0 = time.time()
    ex2 = _SpmdExec(nc2, NCORES)
    staged2 = ex2.stage(in_maps2)
    _log(f"stage2 {time.time() - t0:.1f}s")
    t0 = time.time()
    res2 = ex2.run(staged2)
    _log(f"layer2 run {time.time() - t0:.1f}s")

    if TRACE:
        global LAST_EXEC_NS
        t1 = ex1.time_exec(staged1, TIME_ITERS)
        _log(f"layer1 timed {t1 / 1e6:.2f} ms (marginal of {TIME_ITERS})")
        LAST_EXEC_NS.append(("layer1", t1, t1))
        t2 = ex2.time_exec(staged2, TIME_ITERS)
        _log(f"layer2 timed {t2 / 1e6:.2f} ms (marginal of {TIME_ITERS})")
        LAST_EXEC_NS.append(("layer2", t2, t2))

    z = np.empty((N, H), np.float32)
    for c in range(NCORES):
        sl = orig_of_slot[c]
        valid = sl >= 0
        z[sl[valid]] = res2[c]["out"][valid]
    return z


# revision 13
# speedup vs baseline: 1.2158x; 1.2158x over previous
"""Trainium2 Bass kernel for nn_KGVAE (2-layer BDD RelGraphConv + gaussian sample).

Strategy (8 NeuronCores, SPMD, ONE dispatch):
  - Nodes are range-sharded by id across the 8 cores (12500 each, padded to
    12544 = 98 blocks of 128).  Each core owns all edges whose dst lands in
    its node range, so per-layer aggregates need no cross-core reduction.
  - Within a core, nodes are re-bucketed into 128-node blocks by LPT on
    in-degree so every block sees a near-equal number of incoming edges.
  - Each layer runs in two phases over a DRAM message buffer (bf16):
      Phase A (edges ordered src-chunk-major, relation-minor): batched
        dma_gather(transpose=True) pulls h[src]^T tiles straight into SBUF
        (int16 indices, chunk-relative), one matmul per 128-edge tile
        against the relation's expanded block-diagonal weight, per-edge
        norm scaling fused into the PSUM->SBUF copy, then plain sequential
        DMA of the message tiles to the buffer (no scatter).
      Phase B (per 128-node dst block): batched dma_gather pulls the
        block's messages (msg-chunk-relative indices) as [128, t, F]
        tiles, a one-hot slot matrix built on-device (iota + is_equal)
        aggregates them into PSUM along with the self-loop matmul;
        epilogue adds bias and applies the activation / gaussian head.
  - Between the layers an on-device AllGather exchanges the per-core
    layer-1 activations, so both conv layers + the gaussian head execute
    in a single SPMD dispatch.
  - All bulk tensors are bf16; matmuls bf16 with fp32 PSUM; epilogues fp32.

Self-contained: all shapes derive from the passed-in arrays.
"""

import heapq
import sys
import time
from contextlib import ExitStack

import ml_dtypes
import numpy as np

import concourse.mybir as mybir
import concourse.tile as tile
from concourse import bacc
from concourse import bass2jax as _b2j
from concourse import library_config

P = 128
NCORES = 8
SC = 25088      # src-chunk rows (int16-addressable, multiple of 128)
NSC = 4         # src chunks; NSC*SC = 100352 = padded hsrc rows
MC = 32640      # msgbuf chunk rows (multiple of 128, < 32768)
GA = 4          # phase-A tiles per transpose-gather op (512 idxs; 2 descs/idx)
SCRATCH = 32768  # SWDGE descriptor carveout bytes/partition
NQ = 4          # SWDGE queues
TRACE = False   # set True by a harness to collect exec times
TIME_ITERS = 8
TIME_BURSTS = 3
LAST_EXEC_NS = []

F32 = mybir.dt.float32
BF16 = mybir.dt.bfloat16
I16 = mybir.dt.int16
NPBF16 = ml_dtypes.bfloat16
AF = mybir.ActivationFunctionType
ALU = mybir.AluOpType


def _cdiv(a, b):
    return -(-a // b)


def _expand_bd(W):
    """(R, NB, si, so) block weights -> (R, NB*si, NB*so) dense block-diagonal."""
    R, NB, si, so = W.shape
    out = np.zeros((R, NB * si, NB * so), dtype=np.float32)
    for b in range(NB):
        out[:, b * si:(b + 1) * si, b * so:(b + 1) * so] = W[:, b]
    return out


def _rank_within_group(sorted_keys):
    n = len(sorted_keys)
    if n == 0:
        return np.zeros(0, dtype=np.int64)
    starts = np.searchsorted(sorted_keys, sorted_keys, side="left")
    return np.arange(n, dtype=np.int64) - starts


def _wrap_idx(stream):
    """int idx stream (len % 128 == 0) -> [128, n//16] int16 wrapped layout."""
    n = len(stream)
    w = stream.astype(np.int16).reshape(n // 16, 16).T
    return np.ascontiguousarray(np.tile(w, (8, 1)))


def _plan(src, dst, etype, norm, N, R):
    NPC = N // NCORES
    assert NPC * NCORES == N
    NPAD = _cdiv(NPC, P) * P
    NBLK = NPAD // P
    assert NCORES * NPAD == NSC * SC

    deg = np.bincount(dst, minlength=N)

    # --- LPT assignment of each core's nodes into 128-node blocks ---
    block_of = np.empty(N, np.int32)
    slot_of = np.empty(N, np.int32)
    orig_of_slot = np.full((NCORES, NPAD), -1, np.int64)
    for c in range(NCORES):
        ids = np.arange(c * NPC, (c + 1) * NPC, dtype=np.int64)
        d = deg[ids]
        order = np.argsort(-d, kind="stable")
        heap = [(0, q) for q in range(NBLK)]
        heapq.heapify(heap)
        counts = np.zeros(NBLK, np.int32)
        for i in order:
            while True:
                load, q = heapq.heappop(heap)
                if counts[q] < P:
                    break
            node = ids[i]
            block_of[node] = q
            slot_of[node] = counts[q]
            orig_of_slot[c, q * P + counts[q]] = node
            counts[q] += 1
            heapq.heappush(heap, (load + int(d[i]), q))

    pg = np.empty(N, np.int64)
    for c in range(NCORES):
        ids = np.arange(c * NPC, (c + 1) * NPC, dtype=np.int64)
        pg[ids] = c * NPAD + block_of[ids].astype(np.int64) * P + slot_of[ids]

    core_of_edge = dst // NPC
    eidx_by_core = [np.nonzero(core_of_edge == c)[0] for c in range(NCORES)]

    def layer_tables(use_pg):
        gsrc = [(pg[src[e]] if use_pg else src[e]) for e in eidx_by_core]
        rel = [etype[e] for e in eidx_by_core]
        nrm = [norm[e].reshape(-1).astype(np.float32) for e in eidx_by_core]
        qb = [block_of[dst[e]].astype(np.int64) for e in eidx_by_core]
        pb = [slot_of[dst[e]].astype(np.int64) for e in eidx_by_core]

        cnt = np.zeros((NCORES, NSC * R), np.int64)
        for c in range(NCORES):
            key = (gsrc[c] // SC) * R + rel[c]
            cnt[c] = np.bincount(key, minlength=NSC * R)
        T2 = _cdiv(cnt.max(axis=0), P)          # tiles per (ch, rel)
        A2_tiles = int(T2.sum())
        A2_slots = A2_tiles * P
        goff = np.concatenate([[0], np.cumsum(T2)]) * P

        tiles_rel = []
        for g in range(NSC * R):
            tiles_rel += [(g // R, g % R)] * int(T2[g])

        per_core = []
        for c in range(NCORES):
            key = (gsrc[c] // SC) * R + rel[c]
            order = np.argsort(key, kind="stable")
            rank = _rank_within_group(key[order])
            apos = np.empty(len(key), np.int64)
            apos[order] = goff[key[order]] + rank

            idxA = np.zeros(A2_slots, np.int64)
            normA = np.zeros(A2_slots, np.float32)
            idxA[apos] = gsrc[c] - (gsrc[c] // SC) * SC
            normA[apos] = nrm[c]
            per_core.append(dict(apos=apos, mch=apos // MC,
                                 idxA=idxA, normA=normA))

        NMC = _cdiv(A2_slots, MC)
        cntb = np.zeros((NCORES, NBLK * NMC), np.int64)
        for c in range(NCORES):
            keyb = qb[c] * NMC + per_core[c]["mch"]
            cntb[c] = np.bincount(keyb, minlength=NBLK * NMC)
        TB2 = _cdiv(cntb.max(axis=0), P)        # tiles per (q, mch)
        B2_tiles = int(TB2.sum())
        B2_slots = B2_tiles * P
        boff = np.concatenate([[0], np.cumsum(TB2)]) * P

        for c in range(NCORES):
            pc = per_core[c]
            keyb = qb[c] * NMC + pc["mch"]
            order = np.argsort(keyb, kind="stable")
            rank = _rank_within_group(keyb[order])
            bpos = np.empty(len(keyb), np.int64)
            bpos[order] = boff[keyb[order]] + rank

            idxB = np.zeros(B2_slots, np.int64)
            dstp = np.full(B2_slots, 200.0, np.float32)
            idxB[bpos] = pc["apos"] - pc["mch"] * MC
            dstp[bpos] = pb[c].astype(np.float32)

            per_core[c] = dict(
                idxA_w=_wrap_idx(pc["idxA"]),
                normA_t=np.ascontiguousarray(
                    pc["normA"].reshape(A2_tiles, P).T),
                idxB_w=_wrap_idx(idxB),
                dstp_t=np.ascontiguousarray(dstp.reshape(B2_tiles, P).T),
            )

        return dict(
            T2=T2.reshape(NSC, R), TB2=TB2.reshape(NBLK, NMC),
            A2_tiles=A2_tiles, A2_slots=A2_slots,
            B2_tiles=B2_tiles, B2_slots=B2_slots, NMC=NMC,
            tiles_rel=tiles_rel, per_core=per_core,
        )

    return dict(
        NPC=NPC, NPAD=NPAD, NBLK=NBLK, R=R,
        orig_of_slot=orig_of_slot, pg=pg,
        L1=layer_tables(False), L2=layer_tables(True),
    )


def _emit_layer(nc, tc, pools, consts, io, hsrc, hloc, out_d, msgbuf,
                Wd, loopw_sb, bias_sb, gauss, qrot0):
    """Emit one conv layer (phases A+B). Returns next queue-rotation index."""
    lt, F = io["lt"], io["F"]
    T2, TB2, NMC = lt["T2"], lt["TB2"], lt["NMC"]
    A2_tiles = lt["A2_tiles"]
    B2_tiles = lt["B2_tiles"]
    tiles_rel = lt["tiles_rel"]
    H = 128
    NBLK = TB2.shape[0]
    GB = 8 if F == H else 4

    gpool, spool, wpool, bpool, papool, ptpool, pbpool = pools
    ident, iota_sb = consts["ident"], consts["iota"]
    idxA_sb, normA_sb = io["idxA_sb"], io["normA_sb"]
    idxB_sb, dstp_sb = io["idxB_sb"], io["dstp_sb"]
    eps_bias, one_bias = consts.get("eps_bias"), consts.get("one_bias")
    epsl = io.get("epsl")

    qrot = qrot0

    # ---------------- phase A ----------------
    ops = []
    k = 0
    for ch in range(NSC):
        nt_ch = int(T2[ch].sum())
        t = 0
        while t < nt_ch:
            nt = min(GA, nt_ch - t)
            ops.append((ch, k, nt))
            t += nt
            k += nt
    assert k == A2_tiles

    cur_rel = None
    w_sb = None
    for ch, k0, nt in ops:
        ht = gpool.tile([P, nt * P], BF16, tag="ht")
        nc.gpsimd.dma_gather(
            ht[:].rearrange("p (o n) -> p o n", o=1),
            hsrc[ch * SC:(ch + 1) * SC, :],
            idxA_sb[:, k0 * 8:(k0 + nt) * 8],
            nt * P, nt * P, H, transpose=True, queue_num=qrot,
        )
        qrot = (qrot + 1) % NQ
        stage = spool.tile([P, nt * F], BF16, tag="stage")
        for t in range(nt):
            kt = k0 + t
            r = tiles_rel[kt][1]
            if cur_rel != (ch, r):
                w_sb = wpool.tile([P, F], BF16, tag="w")
                nc.sync.dma_start(out=w_sb[:], in_=Wd[r])
                cur_rel = (ch, r)
            msg_ps = papool.tile([P, F], F32, tag="msg_ps")
            nc.tensor.matmul(out=msg_ps[:], lhsT=ht[:, t * P:(t + 1) * P],
                             rhs=w_sb[:], start=True, stop=True)
            nc.vector.tensor_scalar(
                out=stage[:, t * F:(t + 1) * F], in0=msg_ps[:],
                scalar1=normA_sb[:, kt:kt + 1], scalar2=None, op0=ALU.mult,
            )
        nc.sync.dma_start(
            out=msgbuf[k0 * P:(k0 + nt) * P, :].rearrange(
                "(t p) f -> p t f", p=P),
            in_=stage[:].rearrange("p (t f) -> p t f", f=F),
        )

    tc.strict_bb_all_engine_barrier()

    # ---------------- phase B ----------------
    kb = 0
    for q in range(NBLK):
        TBq = int(TB2[q].sum())
        out_ps = pbpool.tile([P, F], F32, tag="out_ps")
        hl_t = bpool.tile([P, H], BF16, tag="hl")
        nc.sync.dma_start(out=hl_t[:], in_=hloc[q * P:(q + 1) * P, :])
        hlT_ps = ptpool.tile([P, P], BF16, tag="hT_ps")
        nc.tensor.transpose(out=hlT_ps[:], in_=hl_t[:], identity=ident[:])
        hlT_sb = bpool.tile([P, P], BF16, tag="hT")
        nc.scalar.activation(out=hlT_sb[:], in_=hlT_ps[:], func=AF.Copy)
        nc.tensor.matmul(out=out_ps[:], lhsT=hlT_sb[:], rhs=loopw_sb[:],
                         start=True, stop=(TBq == 0))

        done = 0
        for mch in range(NMC):
            ntq = int(TB2[q][mch])
            t0 = 0
            while t0 < ntq:
                nt = min(GB, ntq - t0)
                mblk = bpool.tile([P, nt * F], BF16, tag="mblk")
                nrows = min(MC, lt["A2_slots"] - mch * MC)
                nc.gpsimd.dma_gather(
                    mblk[:].rearrange("p (t f) -> p t f", f=F),
                    msgbuf[mch * MC:mch * MC + nrows, :],
                    idxB_sb[:, (kb + done) * 8:(kb + done + nt) * 8],
                    nt * P, nt * P, F, queue_num=qrot,
                )
                qrot = (qrot + 1) % NQ
                for t in range(nt):
                    ktb = kb + done + t
                    P_t = bpool.tile([P, P], BF16, tag="Pt")
                    nc.vector.tensor_scalar(
                        out=P_t[:], in0=iota_sb[:],
                        scalar1=dstp_sb[:, ktb:ktb + 1], scalar2=None,
                        op0=ALU.is_equal,
                    )
                    nc.tensor.matmul(
                        out=out_ps[:], lhsT=P_t[:],
                        rhs=mblk[:, t * F:(t + 1) * F],
                        start=False, stop=(done + t == TBq - 1),
                    )
                done += nt
                t0 += nt
        kb += TBq

        hb = bpool.tile([P, F], F32, tag="hb")
        nc.vector.tensor_tensor(out=hb[:], in0=out_ps[:], in1=bias_sb[:],
                                op=ALU.add)
        if not gauss:
            hbb = bpool.tile([P, F], BF16, tag="hbb")
            nc.scalar.activation(out=hbb[:], in_=hb[:], func=AF.Relu)
            nc.sync.dma_start(out=out_d[q * P:(q + 1) * P, :], in_=hbb[:])
        else:
            # softplus(x) = relu(x) + ln(1 + exp(-|x|)); sqrt(v) = exp(ln(v)/2)
            H_ = 128
            sq = bpool.tile([P, H_], F32, tag="sq")
            ax = bpool.tile([P, H_], F32, tag="ax")
            nc.scalar.activation(out=ax[:], in_=hb[:, H_:2 * H_], func=AF.Abs)
            nc.scalar.activation(out=ax[:], in_=ax[:], func=AF.Exp, scale=-1.0)
            nc.scalar.activation(out=ax[:], in_=ax[:], func=AF.Ln,
                                 bias=one_bias[:])
            nc.scalar.activation(out=sq[:], in_=hb[:, H_:2 * H_], func=AF.Relu)
            nc.vector.tensor_tensor(out=sq[:], in0=sq[:], in1=ax[:], op=ALU.add)
            nc.scalar.activation(out=sq[:], in_=sq[:], func=AF.Ln,
                                 bias=eps_bias[:])
            nc.scalar.activation(out=sq[:], in_=sq[:], func=AF.Exp, scale=0.5)
            ep = bpool.tile([P, H_], F32, tag="ep")
            nc.sync.dma_start(out=ep[:], in_=epsl[q * P:(q + 1) * P, :])
            z_t = bpool.tile([P, H_], F32, tag="z")
            nc.vector.tensor_tensor(out=z_t[:], in0=sq[:], in1=ep[:], op=ALU.mult)
            nc.vector.tensor_tensor(out=z_t[:], in0=z_t[:], in1=hb[:, :H_],
                                    op=ALU.add)
            nc.sync.dma_start(out=out_d[q * P:(q + 1) * P, :], in_=z_t[:])
    assert kb == B2_tiles
    return qrot


def _build_merged(plan, F2):
    """One SPMD dispatch: layer1 + on-device AllGather + layer2 + head."""
    NPAD, R = plan["NPAD"], plan["R"]
    lt1, lt2 = plan["L1"], plan["L2"]
    H = 128

    nc = bacc.Bacc("TRN2", target_bir_lowering=False, debug=False,
                   num_swdge_queues=NQ, dynamic_dma_scratch_size=SCRATCH)

    hsrc = nc.dram_tensor("hsrc", [NSC * SC, H], BF16, kind="ExternalInput")
    W1d = nc.dram_tensor("W1", [R, H, H], BF16, kind="ExternalInput")
    W2d = nc.dram_tensor("W2", [R, H, F2], BF16, kind="ExternalInput")
    loopw1 = nc.dram_tensor("loopw1", [H, H], BF16, kind="ExternalInput")
    loopw2 = nc.dram_tensor("loopw2", [H, F2], BF16, kind="ExternalInput")
    bias1 = nc.dram_tensor("bias1", [P, H], F32, kind="ExternalInput")
    bias2 = nc.dram_tensor("bias2", [P, F2], F32, kind="ExternalInput")
    hloc = nc.dram_tensor("hloc", [NPAD, H], BF16, kind="ExternalInput")
    iota_d = nc.dram_tensor("iota", [P, P], F32, kind="ExternalInput")
    epsl = nc.dram_tensor("epsl", [NPAD, H], F32, kind="ExternalInput")
    out_d = nc.dram_tensor("out", [NPAD, H], F32, kind="ExternalOutput")

    tabs = {}
    for name, lt in (("1", lt1), ("2", lt2)):
        tabs[name] = dict(
            idxA=nc.dram_tensor(f"idxA{name}", [P, lt["A2_slots"] // 16], I16,
                                kind="ExternalInput"),
            normA=nc.dram_tensor(f"normA{name}", [P, lt["A2_tiles"]], F32,
                                 kind="ExternalInput"),
            idxB=nc.dram_tensor(f"idxB{name}", [P, lt["B2_slots"] // 16], I16,
                                kind="ExternalInput"),
            dstp=nc.dram_tensor(f"dstp{name}", [P, lt["B2_tiles"]], F32,
                                kind="ExternalInput"),
        )

    msg1 = nc.dram_tensor("msg1", [lt1["A2_slots"], H], BF16)
    msg2 = nc.dram_tensor("msg2", [lt2["A2_slots"], F2], BF16)
    h1loc = nc.dram_tensor("h1loc", [NPAD, H], BF16)
    h1full = nc.dram_tensor("h1full", [NCORES * NPAD, H], BF16,
                            addr_space="Shared")

    A2s_cap = max(lt1["A2_slots"], lt2["A2_slots"])
    A2t_cap = max(lt1["A2_tiles"], lt2["A2_tiles"])
    B2s_cap = max(lt1["B2_slots"], lt2["B2_slots"])
    B2t_cap = max(lt1["B2_tiles"], lt2["B2_tiles"])

    with tile.TileContext(nc) as tc, ExitStack() as ctx:
        nc.gpsimd.load_library(library_config.mlp)
        const = ctx.enter_context(tc.tile_pool(name="const", bufs=1))
        gpool = ctx.enter_context(tc.tile_pool(name="gpool", bufs=3))
        spool = ctx.enter_context(tc.tile_pool(name="spool", bufs=3))
        wpool = ctx.enter_context(tc.tile_pool(name="wpool", bufs=3))
        bpool = ctx.enter_context(tc.tile_pool(name="bpool", bufs=4))
        papool = ctx.enter_context(tc.tile_pool(name="papool", bufs=3, space="PSUM"))
        ptpool = ctx.enter_context(tc.tile_pool(name="ptpool", bufs=2, space="PSUM"))
        pbpool = ctx.enter_context(tc.tile_pool(name="pbpool", bufs=2, space="PSUM"))
        pools = (gpool, spool, wpool, bpool, papool, ptpool, pbpool)

        ident = const.tile([P, P], BF16)
        iota_sb = const.tile([P, P], F32)
        loopw1_sb = const.tile([H, H], BF16)
        loopw2_sb = const.tile([H, F2], BF16)
        bias1_sb = const.tile([P, H], F32)
        bias2_sb = const.tile([P, F2], F32)
        idxA_sb = const.tile([P, A2s_cap // 16], I16)
        normA_sb = const.tile([P, A2t_cap], F32)
        idxB_sb = const.tile([P, B2s_cap // 16], I16)
        dstp_sb = const.tile([P, B2t_cap], F32)
        nc.sync.dma_start(out=iota_sb[:], in_=iota_d[:])
        nc.sync.dma_start(out=loopw1_sb[:], in_=loopw1[:])
        nc.sync.dma_start(out=loopw2_sb[:], in_=loopw2[:])
        nc.sync.dma_start(out=bias1_sb[:], in_=bias1[:])
        nc.sync.dma_start(out=bias2_sb[:], in_=bias2[:])
        from concourse.masks import make_identity
        make_identity(nc, ident[:])
        eps_bias = const.tile([P, 1], F32)
        nc.vector.memset(eps_bias[:], 1e-8)
        one_bias = const.tile([P, 1], F32)
        nc.vector.memset(one_bias[:], 1.0)
        consts = dict(ident=ident, iota=iota_sb, eps_bias=eps_bias,
                      one_bias=one_bias)

        def load_tables(name, lt):
            t = tabs[name]
            nc.sync.dma_start(out=idxA_sb[:, :lt["A2_slots"] // 16],
                              in_=t["idxA"][:])
            nc.sync.dma_start(out=normA_sb[:, :lt["A2_tiles"]],
                              in_=t["normA"][:])
            nc.sync.dma_start(out=idxB_sb[:, :lt["B2_slots"] // 16],
                              in_=t["idxB"][:])
            nc.sync.dma_start(out=dstp_sb[:, :lt["B2_tiles"]],
                              in_=t["dstp"][:])

        def io_for(lt, F, with_eps):
            d = dict(lt=lt, F=F,
                     idxA_sb=idxA_sb[:, :lt["A2_slots"] // 16],
                     normA_sb=normA_sb[:, :lt["A2_tiles"]],
                     idxB_sb=idxB_sb[:, :lt["B2_slots"] // 16],
                     dstp_sb=dstp_sb[:, :lt["B2_tiles"]])
            if with_eps:
                d["epsl"] = epsl
            return d

        load_tables("1", lt1)
        qrot = _emit_layer(nc, tc, pools, consts, io_for(lt1, H, False),
                           hsrc, hloc, h1loc, msg1, W1d, loopw1_sb,
                           bias1_sb, gauss=False, qrot0=0)

        tc.strict_bb_all_engine_barrier()
        nc.gpsimd.collective_compute(
            "AllGather", mybir.AluOpType.bypass,
            replica_groups=[list(range(NCORES))],
            ins=[h1loc[:]], outs=[h1full[:]],
        )
        load_tables("2", lt2)
        tc.strict_bb_all_engine_barrier()

        _emit_layer(nc, tc, pools, consts, io_for(lt2, F2, True),
                    h1full, h1loc, out_d, msg2, W2d, loopw2_sb,
                    bias2_sb, gauss=True, qrot0=qrot)

    nc.compile()
    return nc


def _log(msg):
    print(f"[kernel] {msg}", file=sys.stderr, flush=True)


class _SpmdExec:
    """Compile a bass module into one persistent sharded PJRT executable.

    Inputs are staged onto the 8 devices once (device_put, untimed); runs
    reuse the staged arrays."""

    def __init__(self, nc, n_cores):
        import jax

        _b2j.install_neuronx_cc_hook()
        self.nc = nc
        self.n_cores = n_cores
        partition_name = (
            nc.partition_id_tensor.name if nc.partition_id_tensor else None
        )
        in_names, out_names, out_avals, zero_info = [], [], [], []
        for alloc in nc.m.functions[0].allocations:
            if not isinstance(alloc, mybir.MemoryLocationSet):
                continue
            name = alloc.memorylocations[0].name
            if alloc.kind == "ExternalInput":
                if name != partition_name:
                    in_names.append(name)
            elif alloc.kind == "ExternalOutput":
                out_names.append(name)
                shape = tuple(alloc.tensor_shape)
                dtype = mybir.dt.np(alloc.dtype)
                out_avals.append(jax.core.ShapedArray(shape, dtype))
                zero_info.append((shape, dtype))
        assert nc.dbg_addr is None, "build with debug=False"
        self.in_names = list(in_names)
        self.out_names = out_names
        n_params = len(in_names)
        n_outs = len(out_names)
        in_names = in_names + out_names
        if partition_name is not None:
            in_names.append(partition_name)

        def _body(*args):
            operands = list(args)
            if partition_name is not None:
                operands.append(_b2j.partition_id_tensor())
            outs = _b2j._bass_exec_p.bind(
                *operands,
                out_avals=tuple(out_avals),
                in_names=tuple(in_names),
                out_names=tuple(out_names),
                lowering_input_output_aliases=(),
                sim_require_finite=True,
                sim_require_nnan=True,
                nc=nc,
            )
            return tuple(outs)

        from jax.experimental.shard_map import shard_map
        from jax.sharding import Mesh, NamedSharding, PartitionSpec

        devices = jax.devices()[:n_cores]
        mesh = Mesh(np.asarray(devices), ("core",))
        self.mesh = mesh
        self.sharding = NamedSharding(mesh, PartitionSpec("core"))
        in_specs = (PartitionSpec("core"),) * (n_params + n_outs)
        out_specs = (PartitionSpec("core"),) * n_outs
        donate = tuple(range(n_params, n_params + n_outs))
        self.sharded = jax.jit(
            shard_map(_body, mesh=mesh, in_specs=in_specs,
                      out_specs=out_specs, check_rep=False),
            donate_argnums=donate, keep_unused=True,
        )
        import jax.numpy as jnp

        def _mk():
            return tuple(
                jnp.zeros((n_cores * s[0], *s[1:]), d) for s, d in zero_info
            )

        self._mkzeros = jax.jit(
            _mk, out_shardings=(self.sharding,) * n_outs)
        self.out_avals = out_avals

    def stage(self, in_maps):
        import jax

        staged = []
        for name in self.in_names:
            arr = np.concatenate(
                [np.asarray(m[name]) for m in in_maps], axis=0)
            staged.append(jax.device_put(arr, self.sharding))
        jax.block_until_ready(staged)
        return staged

    def run(self, staged):
        import jax

        zs = self._mkzeros()
        outs = self.sharded(*staged, *zs)
        jax.block_until_ready(outs)
        res = []
        for c in range(self.n_cores):
            d = {}
            for i, name in enumerate(self.out_names):
                g = np.asarray(outs[i])
                d[name] = g.reshape(self.n_cores, *self.out_avals[i].shape)[c]
            res.append(d)
        for o in outs:
            o.delete()
        return res

    def time_exec(self, staged, iters, bursts):
        """Steady-state per-dispatch device time: per burst, one warm call
        (T1) then `iters` pipelined calls; marginal = (Tk - T1)/(k-1)
        excludes the client<->terminal round-trip latency. Min over bursts
        rejects contention noise."""
        import jax

        best = None
        for b in range(bursts):
            zs_all = [self._mkzeros() for _ in range(iters + 1)]
            jax.block_until_ready(zs_all)
            t0 = time.perf_counter()
            out1 = self.sharded(*staged, *zs_all[0])
            jax.block_until_ready(out1)
            t1 = time.perf_counter()
            outs = [self.sharded(*staged, *zs_all[1 + i]) for i in range(iters)]
            jax.block_until_ready(outs)
            t2 = time.perf_counter()
            single = t1 - t0
            marginal = (t2 - t1 - single) / max(iters - 1, 1)
            _log(f"  burst {b}: single {single * 1e3:.2f} ms, +{iters} calls "
                 f"{(t2 - t1) * 1e3:.2f} ms -> marginal {marginal * 1e3:.2f} ms")
            best = marginal if best is None else min(best, marginal)
            for o in outs:
                for x in o:
                    x.delete()
            for x in out1:
                x.delete()
        return int(best * 1e9)


def kernel(node_ids, src, dst, etype, norm, emb, W1, loop1, b1, W2, loop2, b2, eps):
    node_ids = np.asarray(node_ids).astype(np.int64)
    src = np.asarray(src).astype(np.int64)
    dst = np.asarray(dst).astype(np.int64)
    etype = np.asarray(etype).astype(np.int64)
    norm = np.asarray(norm, np.float32)
    emb = np.asarray(emb, np.float32)
    W1 = np.asarray(W1, np.float32)
    loop1 = np.asarray(loop1, np.float32)
    b1 = np.asarray(b1, np.float32)
    W2 = np.asarray(W2, np.float32)
    loop2 = np.asarray(loop2, np.float32)
    b2 = np.asarray(b2, np.float32)
    eps = np.asarray(eps, np.float32)

    N, H = emb.shape
    R = W1.shape[0]
    F2 = W2.shape[1] * W2.shape[3]
    assert H == 128

    h0 = emb[node_ids]
    t0 = time.time()
    plan = _plan(src, dst, etype, norm, N, R)
    _log(f"plan {time.time() - t0:.1f}s "
         f"L1 A2={plan['L1']['A2_tiles']} B2={plan['L1']['B2_tiles']} "
         f"L2 A2={plan['L2']['A2_tiles']} B2={plan['L2']['B2_tiles']}")
    NPAD = plan["NPAD"]
    orig_of_slot = plan["orig_of_slot"]

    h0b = np.zeros((NSC * SC, H), NPBF16)
    h0b[:N] = h0.astype(NPBF16)
    W1bd = _expand_bd(W1).astype(NPBF16)
    W2bd = _expand_bd(W2).astype(NPBF16)
    iota = np.tile(np.arange(P, dtype=np.float32), (P, 1))
    bias1b = np.tile(b1.astype(np.float32), (P, 1))
    bias2b = np.tile(b2.astype(np.float32), (P, 1))

    hloc_c, eps_c = [], []
    for c in range(NCORES):
        sl = orig_of_slot[c]
        valid = sl >= 0
        hl = np.zeros((NPAD, H), NPBF16)
        hl[valid] = h0b[sl[valid]]
        ev = np.zeros((NPAD, H), np.float32)
        ev[valid] = eps[sl[valid]]
        hloc_c.append(hl)
        eps_c.append(ev)

    t0 = time.time()
    nc = _build_merged(plan, F2)
    _log(f"build {time.time() - t0:.1f}s")
    in_maps = []
    for c in range(NCORES):
        p1 = plan["L1"]["per_core"][c]
        p2 = plan["L2"]["per_core"][c]
        in_maps.append(dict(
            hsrc=h0b, W1=W1bd, W2=W2bd,
            loopw1=loop1.astype(NPBF16), loopw2=loop2.astype(NPBF16),
            bias1=bias1b, bias2=bias2b, hloc=hloc_c[c], iota=iota,
            epsl=eps_c[c],
            idxA1=p1["idxA_w"], normA1=p1["normA_t"],
            idxB1=p1["idxB_w"], dstp1=p1["dstp_t"],
            idxA2=p2["idxA_w"], normA2=p2["normA_t"],
            idxB2=p2["idxB_w"], dstp2=p2["dstp_t"],
        ))
    t0 = time.time()
    ex = _SpmdExec(nc, NCORES)
    staged = ex.stage(in_maps)
    _log(f"stage {time.time() - t0:.1f}s")
    t0 = time.time()
    res = ex.run(staged)
    _log(f"run {time.time() - t0:.1f}s")

    if TRACE:
        global LAST_EXEC_NS
        t1 = ex.time_exec(staged, TIME_ITERS, TIME_BURSTS)
        _log(f"merged timed {t1 / 1e6:.2f} ms")
        LAST_EXEC_NS.append(("merged", t1, t1))

    z = np.empty((N, H), np.float32)
    for c in range(NCORES):
        sl = orig_of_slot[c]
        valid = sl >= 0
        z[sl[valid]] = res[c]["out"][valid]
    return z
